# revision 1
# baseline (speedup 1.0000x reference)
"""Trainium2 Bass kernel for nn_KANStressPredictor.

Computes, per element-triple (s0, s1, s2) of `strain` [B, T, 3]:
    c00 = 2*s0+1, c11 = 2*s1+1, c01 = s2          (C = 2E + I, sym 2x2)
    t1, t2 = eigenvalues of C   (t = mean -/+ rad)
    out0, out1 = (sqrt(t_i) * det^(-1/6))^ki0
    out2       = ki1 * 0.5 * log(det)

Key algebraic reductions used here:
    mean = s0+s1+1,  rad^2 = (s0-s1)^2 + s2^2,  t_i = (s0+s1) -/+ rad + 1
    det  = t1*t2  =>  log(det) = log(t1) + log(t2)   (never materialized)
    out_i = exp(ki0/3 * (log(t_i) - 0.5*log(t_other)));  out2 from L = l1+l2
    rad  = exp(0.5*ln(rad^2))  -- keeps ACT in one table set (ln/exp only)

Sharding: pure data-parallel over the batch dim across 8 cores. Each core's
shard is viewed as [128, F] (partition-contiguous), processed in free-dim
chunks with interleaved-triple strided access patterns.
"""

import sys

for _p in ("/opt/trn_rl_repo",):
    if _p not in sys.path:
        sys.path.insert(0, _p)

import numpy as np

import concourse.bacc as bacc
import concourse.bass as bass
import concourse.tile as tile
from concourse import mybir
from concourse.bass_utils import run_bass_kernel_spmd

N_CORES = 8
P = 128

_cache: dict = {}


def _build(ki0: float, ki1: float, F: int, chunk_triples: int, reps: int = 1):
    """Build + compile the Bass program for one core's [P, F] shard.

    reps > 1 repeats the whole pipeline (same input/output) for benchmarking:
    marginal time per rep = steady-state exec time with dispatch cancelled.
    """
    key = (ki0, ki1, F, chunk_triples, reps)
    if key in _cache:
        return _cache[key]

    f32 = mybir.dt.float32
    AF = mybir.ActivationFunctionType
    Add = mybir.AluOpType.add
    Sub = mybir.AluOpType.subtract
    Mult = mybir.AluOpType.mult

    CT = chunk_triples
    CE = CT * 3  # elems per chunk per partition
    assert F % CE == 0
    n_chunks = F // CE

    nc = bacc.Bacc("TRN2", target_bir_lowering=False, debug=False)
    in_ap = nc.dram_tensor("strain", [P, F], f32, kind="ExternalInput").ap()
    out_ap = nc.dram_tensor("out", [P, F], f32, kind="ExternalOutput").ap()

    with tile.TileContext(nc) as tc:
        with (
            tc.tile_pool(name="io", bufs=2) as iop,
            tc.tile_pool(name="pl", bufs=3) as pl,
        ):
            for ci in range(n_chunks * reps):
                ci = ci % n_chunks
                sl = bass.ts(ci, CE)
                I = iop.tile([P, CE], f32, name="in", tag="in")
                nc.sync.dma_start(I[:], in_ap[:, sl])
                I3 = I[:].rearrange("p (n k) -> p k n", k=3)
                a, b, c = I3[:, 0], I3[:, 1], I3[:, 2]

                O = iop.tile([P, CE], f32, name="out", tag="out")
                Opair = O[:].rearrange("p (n k) -> p n k", k=3)[:, :, 0:2]
                Oc = O[:].rearrange("p (n k) -> p k n", k=3)[:, 2]

                def T(tag, width=CT):
                    return pl.tile([P, width], f32, name=tag, tag=tag)[:]

                s = T("s")
                nc.vector.tensor_add(s, a, b)  # s0+s1
                u = T("u")
                nc.vector.tensor_sub(u, a, b)  # s0-s1
                q = T("q")
                nc.scalar.activation(q, c, AF.Square)  # s2^2
                nc.scalar.activation(u, u, AF.Square)  # (s0-s1)^2, in place
                r2 = T("r2")
                nc.vector.tensor_add(r2, u, q)  # rad^2
                nc.scalar.activation(r2, r2, AF.Ln)  # in place
                rad = T("rad")
                nc.scalar.activation(rad, r2, AF.Exp, scale=0.5)  # sqrt(rad^2)

                D = T("D", 2 * CT)  # (d1, d2) interleaved pairs
                Dp = D[:].rearrange("p (n k) -> p n k", k=2)
                nc.vector.scalar_tensor_tensor(
                    Dp[:, :, 0], rad, -1.0, s, Mult, Add
                )  # d1 = s - rad
                nc.vector.tensor_add(Dp[:, :, 1], s, rad)  # d2 = s + rad
                # l = ln(d + 1) for both eigenvalues in one pass
                nc.scalar.activation(D[:], D[:], AF.Ln, bias=1.0)
                l3 = D[:].rearrange("p (n k) -> p n k", k=2)
                l1, l2 = l3[:, :, 0], l3[:, :, 1]
                lswap = l3[:, :, ::-1]
                L = T("L")
                nc.vector.tensor_add(L, l1, l2)  # log(det)
                # w_i = l_i - 0.5*l_other; out_i = exp(ki0/3 * w_i)
                W = T("W", 2 * CT)
                Wp = W[:].rearrange("p (n k) -> p n k", k=2)
                nc.vector.scalar_tensor_tensor(Wp, lswap, -0.5, l3, Mult, Add)
                nc.scalar.activation(Opair, Wp, AF.Exp, scale=ki0 / 3.0)
                nc.scalar.mul(Oc, L, ki1 * 0.5)

                nc.sync.dma_start(out_ap[:, sl], O[:])

    nc.compile()
    _cache[key] = nc
    return nc


def _run(strain: np.ndarray, ki0: float, ki1: float, trace: bool = False,
         chunk_triples: int = 1024):
    B, T, C = strain.shape
    assert C == 3 and B % N_CORES == 0
    Bs = B // N_CORES
    elems = Bs * T * C
    assert elems % P == 0
    F = elems // P
    assert (F // 3) % chunk_triples == 0

    nc = _build(float(ki0), float(ki1), F, chunk_triples)

    flat = np.ascontiguousarray(strain, dtype=np.float32).reshape(N_CORES, P, F)
    in_maps = [{"strain": flat[i]} for i in range(N_CORES)]
    res = run_bass_kernel_spmd(nc, in_maps, list(range(N_CORES)), trace=trace)
    out = np.stack([np.asarray(res.results[i]["out"]) for i in range(N_CORES)])
    out = out.reshape(B, T, C).astype(np.float32, copy=False)
    return out, res


def kernel(strain: np.ndarray, ki0, ki1) -> np.ndarray:
    out, _ = _run(np.asarray(strain), float(np.asarray(ki0)), float(np.asarray(ki1)))
    return out



# revision 2
# speedup vs baseline: 1.0557x; 1.0557x over previous
"""Trainium2 Bass kernel for nn_KANStressPredictor (planar bf16, 8-core DP).

Math per strain triple (s0, s1, s2), with C = 2E + I symmetric 2x2:
    t12 = (s0+s1) -/+ rad,  rad = sqrt((s0-s1)^2 + s2^2)
    l_i = ln(t_i + 1)                       (eigenvalues are t_i + 1)
    out_i = exp(ki0/3 * (l_i - 0.5*l_other))     i in {0,1}
    out_2 = ki1 * 0.5 * (l1 + l2)

Implementation notes:
  * bf16 end-to-end (host casts): halves HBM traffic vs f32 and unlocks
    DVE 2x (tensor_tensor) / 4x (tensor_scalar) perf modes.
  * Planar per-chunk layout [a|b|c] per partition row (host pre-transposes)
    so every engine op is a dense step-1 slice; strided access would force
    the DVE into 1x mode.
  * rad via exp(0.5*ln(r2)) keeps all activations in the single
    natural_log_exp_and_others table set; one explicit LoadActFuncSet up
    front means zero table reloads (a greedy chooser otherwise ping-pongs
    exp_and_others/natural_log every chunk, ~2.7us per reload).
  * No scalar_tensor_tensor (no DVE accel uops - always 1x).  The affine
    combos are restructured as h = 0.5*l12 (one 4x tensor_scalar), then
    w_i = l_i - h_other and out2 = h1 + h2 as plain 2x tensor_tensors.
  * GPSIMD (Pool) takes c^2 and out2 off the DVE.
  * Output DMAs issue from the ACT sequencer (qActDynamicHW ring) so in-
    and out-streams ride separate HWDGE FIFOs instead of serializing on
    qSPDynamicHW.

Sharding: pure data-parallel over the batch dim across 8 cores; host
reassembles.  ki0/ki1 are compile-time constants (cached per value).
"""

import sys

for _p in ("/opt/trn_rl_repo",):
    if _p not in sys.path:
        sys.path.insert(0, _p)

import numpy as np
import ml_dtypes

import concourse.bacc as bacc
import concourse.bass as bass
import concourse.tile as tile
from concourse import mybir
from concourse.bass_utils import run_bass_kernel_spmd

N_CORES = 8
P = 128
BF16 = ml_dtypes.bfloat16

# Tuned on hardware (reps-marginal benchmarks):
CT_DEFAULT = 1024      # triples per chunk -> per-partition chunk [a|b|c]
IO_BUFS = 4
WK_BUFS = 3
OUT_SC = True          # out-DMA on the ACT HWDGE ring

_cache: dict = {}


def _lnexp_set_id(nc) -> int:
    try:
        from concourse.hw_specs import get_activation_tables

        return list(get_activation_tables(nc.m.arch)).index(
            "natural_log_exp_and_others"
        )
    except Exception:
        return 6


def _build(ki0: float, ki1: float, F: int, CT: int, reps: int = 1,
           out_sc: bool = OUT_SC):
    key = (ki0, ki1, F, CT, reps, out_sc)
    if key in _cache:
        return _cache[key]

    bf16 = mybir.dt.bfloat16
    AF = mybir.ActivationFunctionType
    CE = 3 * CT
    assert F % CE == 0
    n_chunks = F // CE

    nc = bacc.Bacc("TRN2", target_bir_lowering=False, debug=False)
    in_ap = nc.dram_tensor("strain", [P, F], bf16, kind="ExternalInput").ap()
    out_ap = nc.dram_tensor("out", [P, F], bf16, kind="ExternalOutput").ap()

    nc.scalar.add_instruction(
        mybir.InstLoadActFuncSet(
            name=nc.get_next_instruction_name(),
            act_func_set_id=_lnexp_set_id(nc),
            engine=mybir.EngineType.Activation,
        )
    )

    with tile.TileContext(nc) as tc:
        with (
            tc.tile_pool(name="io", bufs=IO_BUFS) as iop,
            tc.tile_pool(name="wk", bufs=WK_BUFS) as wk,
        ):
            for ci in range(n_chunks * reps):
                ci = ci % n_chunks
                sl = bass.ts(ci, CE)
                out_eng = nc.scalar if out_sc else nc.sync
                I = iop.tile([P, CE], bf16, name="in", tag="in")
                nc.sync.dma_start(I[:], in_ap[:, sl])
                a, b, c = I[:, 0:CT], I[:, CT : 2 * CT], I[:, 2 * CT : 3 * CT]

                s = wk.tile([P, CT], bf16, name="s", tag="s")[:]
                u = wk.tile([P, CT], bf16, name="u", tag="u")[:]
                c2 = wk.tile([P, CT], bf16, name="c2", tag="c2")[:]
                rad = wk.tile([P, CT], bf16, name="rad", tag="rad")[:]

                nc.vector.tensor_add(s, a, b)        # s0+s1
                nc.vector.tensor_sub(u, a, b)        # s0-s1
                nc.gpsimd.tensor_mul(c2, c, c)       # s2^2   (Pool)
                nc.vector.tensor_mul(u, u, u)        # (s0-s1)^2, in place
                nc.vector.tensor_add(c2, u, c2)      # r2, in place
                nc.scalar.activation(c2, c2, AF.Ln)              # ln(r2)
                nc.scalar.activation(rad, c2, AF.Exp, scale=0.5)  # rad

                T12 = wk.tile([P, 2 * CT], bf16, name="t12", tag="t12")[:]
                nc.vector.tensor_sub(T12[:, 0:CT], s, rad)   # t1
                nc.vector.tensor_add(T12[:, CT:], s, rad)    # t2
                L12 = wk.tile([P, 2 * CT], bf16, name="l12", tag="l12")[:]
                nc.scalar.activation(L12, T12, AF.Ln, bias=1.0)  # ln(t+1)
                H = wk.tile([P, 2 * CT], bf16, name="h", tag="h")[:]
                nc.vector.tensor_scalar_mul(H, L12, 0.5)
                l1, l2 = L12[:, 0:CT], L12[:, CT:]
                h1, h2 = H[:, 0:CT], H[:, CT:]

                O = iop.tile([P, CE], bf16, name="out", tag="out")
                W12 = T12  # reuse
                nc.vector.tensor_sub(W12[:, 0:CT], l1, h2)   # w1
                nc.vector.tensor_sub(W12[:, CT:], l2, h1)    # w2
                nc.scalar.activation(
                    O[:, 0 : 2 * CT], W12, AF.Exp, scale=ki0 / 3.0
                )  # out0, out1
                o2 = O[:, 2 * CT : 3 * CT]
                nc.gpsimd.tensor_add(o2, h1, h2)             # out2 (Pool)
                if ki1 != 1.0:
                    nc.vector.tensor_scalar_mul(o2, o2, ki1)

                out_eng.dma_start(out_ap[:, sl], O[:])

    nc.compile()
    _cache[key] = nc
    return nc


def _prep(strain: np.ndarray, CT: int) -> np.ndarray:
    """[B, T, 3] f32 -> [N_CORES, P, F] bf16 planar chunks."""
    B, T, C = strain.shape
    F = B * T * C // (N_CORES * P)
    n_chunks = F // (3 * CT)
    x = np.ascontiguousarray(strain, dtype=np.float32).astype(BF16)
    x = x.reshape(N_CORES, P, n_chunks, CT, 3)
    x = x.transpose(0, 1, 2, 4, 3)
    return np.ascontiguousarray(x).reshape(N_CORES, P, F)


def _unprep(out: np.ndarray, B: int, T: int, CT: int) -> np.ndarray:
    F = out.shape[-1]
    n_chunks = F // (3 * CT)
    y = out.reshape(N_CORES, P, n_chunks, 3, CT)
    y = y.transpose(0, 1, 2, 4, 3)
    return np.ascontiguousarray(y).astype(np.float32).reshape(B, T, 3)


def _run(strain: np.ndarray, ki0: float, ki1: float, trace: bool = False,
         CT: int = CT_DEFAULT):
    B, T, C = strain.shape
    assert C == 3 and B % N_CORES == 0
    F = (B // N_CORES) * T * C // P
    assert F % (3 * CT) == 0

    nc = _build(float(ki0), float(ki1), F, CT)
    flat = _prep(strain, CT)
    in_maps = [{"strain": flat[i]} for i in range(N_CORES)]
    res = run_bass_kernel_spmd(nc, in_maps, list(range(N_CORES)), trace=trace)
    out = np.stack([np.asarray(res.results[i]["out"]) for i in range(N_CORES)])
    return _unprep(out, B, T, CT), res


def kernel(strain: np.ndarray, ki0, ki1) -> np.ndarray:
    out, _ = _run(
        np.asarray(strain), float(np.asarray(ki0)), float(np.asarray(ki1))
    )
    return out
